# revision 1
# baseline (speedup 1.0000x reference)
"""Multi-head causal attention (B=2, T=2048, D=2048, H=16, dk=128) on 8 TRN2 NeuronCores.

Strategy (tensor-parallel over heads, 2 heads/core):
  - Host prep: transpose x -> xT [D, B*T], slice+transpose Wq/Wk/Wv per core
    ([D, 256] each), full Wo.T. All fed as float32 (device treats as f32r).
  - Per core: QT/KT = W.T-slices^T @ ... computed as PE matmuls producing
    Q^T/K^T layouts [dk, tokens]; V natural [tokens, dk].
  - Attention with TRANSPOSED scores: S^T[kk, q] chunks per kk-tile so the
    exp'd probabilities land directly in P^T layout (no PE transposes of P).
    No max-subtraction (scores are ~N(0,1); exp cannot overflow). Softmax
    denominator via ones[128,128] matmul over P^T partitions (result arrives
    pre-broadcast to all partitions); 1/d via DVE reciprocal_approx_fast,
    applied in the ctx PSUM->SBUF copy.
  - ctx^T [dk, tokens] per head -> AllToAll (2 MB/core) so each core gets all
    16 heads' ctx^T for its 512-token slice -> local Wo projection ->
    out [512, 2048]. Wo accumulation is split even/odd c-tiles: evens (ready
    after the first AllToAll) run during the second collective, partials spill
    to SBUF, odds accumulate in fresh PSUM, DVE add merges on the way out.
  - Host: concatenate the 8 row-slices.
  - Engine-queue placement is deliberate: collectives + cm loads on gpsimd,
    csb stores + half the woT stream on sync, exps + other half on scalar —
    a slot-waiting prefetch DMA must never sit ahead of critical work in an
    engine FIFO.

Everything matmul-facing uses dtype float32r: full TensorE rate (1 cyc/row,
same as bf16) at ~13-bit mantissa accuracy (~1.5e-4 per matmul).
"""

import math
import numpy as np
from contextlib import ExitStack

import concourse.tile as tile
import concourse.mybir as mybir
from concourse import bacc
from concourse.bass_utils import run_bass_kernel_spmd

B, T, D = 2, 2048, 2048
H, DK = 16, 128
NCORES = 8
HL = H // NCORES            # 2 heads per core
OC = HL * DK                # 256 out dims per core
TT = B * T                  # 4096 flat tokens
TCHUNK = 512
NTC = TT // TCHUNK          # 8 token chunks (proj)
NKT = D // 128              # 16 contraction tiles
NQC = T // TCHUNK           # 4 q-chunks per batch
SCALE = 1.0 / math.sqrt(DK)
F32 = mybir.dt.float32
MDT = mybir.dt.float32r
MASK_VAL = -1e30

_CACHE = {}


def build():
    nc = bacc.Bacc("TRN2", target_bir_lowering=False, debug=False, num_devices=NCORES)

    xt_d = nc.dram_tensor("xt", [D, TT], MDT, kind="ExternalInput")
    wqt_d = nc.dram_tensor("wqt", [D, OC], MDT, kind="ExternalInput")
    wkt_d = nc.dram_tensor("wkt", [D, OC], MDT, kind="ExternalInput")
    wvt_d = nc.dram_tensor("wvt", [D, OC], MDT, kind="ExternalInput")
    wot_d = nc.dram_tensor("wot", [D, D], MDT, kind="ExternalInput")
    out_d = nc.dram_tensor("out", [TT // NCORES, D], F32, kind="ExternalOutput")

    with tile.TileContext(nc) as tc, ExitStack() as ctx:
        psum = ctx.enter_context(tc.tile_pool(name="ps", bufs=8, space="PSUM"))
        dram = ctx.enter_context(tc.tile_pool(name="dram", bufs=1, space="DRAM"))
        persist = ctx.enter_context(tc.tile_pool(name="persist", bufs=1))
        small = ctx.enter_context(tc.tile_pool(name="small", bufs=2))

        # ---- persistent SBUF: QT/KT [128, HL*TT] (cols: h*TT + flat_tok), V [128, 32*256]
        QT = persist.tile([128, HL * TT], MDT, name="QTs")
        KT = persist.tile([128, HL * TT], MDT, name="KTs")
        Vs = persist.tile([128, (TT // 128) * OC], MDT, name="Vs")

        # ---- identity (needed by proj-phase V transposes)
        ident = persist.tile([128, 128], MDT, name="ident")
        with tc.tile_pool(name="cstage0", bufs=1) as cstage0:
            ident_f = cstage0.tile([128, 128], F32, name="ident_f")
            from concourse.masks import make_identity
            make_identity(nc, ident_f[:])
            nc.vector.tensor_copy(ident[:], ident_f[:])

        # =================== Phase 1: QKV projections ===================
        with tc.tile_pool(name="proj", bufs=1) as projp, \
             tc.tile_pool(name="xtp", bufs=6) as xtp:
            wq_sb = projp.tile([128, NKT * OC], MDT, name="wq_sb")
            wk_sb = projp.tile([128, NKT * OC], MDT, name="wk_sb")
            wv_sb = projp.tile([128, NKT * OC], MDT, name="wv_sb")
            # split weight loads into k-groups so the first matmuls start early
            for kg in range(0, NKT, 4):
                for w_sb, w_d in ((wq_sb, wqt_d), (wk_sb, wkt_d), (wv_sb, wvt_d)):
                    nc.sync.dma_start(
                        out=w_sb[:, kg * OC:(kg + 4) * OC].rearrange("p (kt o) -> p kt o", kt=4),
                        in_=w_d.ap()[kg * 128:(kg + 4) * 128, :].rearrange("(kt p) o -> p kt o", p=128),
                    )

            for tcx in range(NTC):
                xts = []
                for k in range(NKT):
                    xt = xtp.tile([128, TCHUNK], MDT, tag="xt", name=f"xt_{tcx}_{k}")
                    nc.gpsimd.dma_start(
                        out=xt[:],
                        in_=xt_d.ap()[k * 128:(k + 1) * 128, tcx * TCHUNK:(tcx + 1) * TCHUNK],
                    )
                    xts.append(xt)

                qp = [psum.tile([128, 512], F32, tag="mm", name=f"qp{tcx}_{o}") for o in range(HL)]
                kp = [psum.tile([128, 512], F32, tag="mm", name=f"kp{tcx}_{o}") for o in range(HL)]
                vp = [psum.tile([128, 512], F32, tag="mm", name=f"vp{tcx}_{o}") for o in range(HL)]
                for k in range(NKT):
                    st, sp = (k == 0), (k == NKT - 1)
                    for o in range(HL):
                        nc.tensor.matmul(qp[o][:], wq_sb[:, k * OC + o * 128: k * OC + (o + 1) * 128],
                                         xts[k][:], start=st, stop=sp)
                        nc.tensor.matmul(kp[o][:], wk_sb[:, k * OC + o * 128: k * OC + (o + 1) * 128],
                                         xts[k][:], start=st, stop=sp)
                        nc.tensor.matmul(vp[o][:], wv_sb[:, k * OC + o * 128: k * OC + (o + 1) * 128],
                                         xts[k][:], start=st, stop=sp)
                for o in range(HL):
                    dst = slice(o * TT + tcx * TCHUNK, o * TT + (tcx + 1) * TCHUNK)
                    nc.scalar.copy(QT[:, dst], qp[o][:])
                    nc.scalar.copy(KT[:, dst], kp[o][:])
                # V^T -> V via PE transposes (V^T psum -> sbuf -> transpose -> V)
                for o in range(HL):
                    vts = small.tile([128, 512], MDT, tag="vts", name=f"vts{tcx}_{o}")
                    nc.vector.tensor_copy(vts[:], vp[o][:])
                    for t4 in range(4):
                        t32 = tcx * 4 + t4
                        vtp = psum.tile([128, 128], MDT, tag="mm", name=f"vtp{tcx}_{o}_{t4}")
                        nc.tensor.transpose(vtp[:], vts[:, t4 * 128:(t4 + 1) * 128], ident[:])
                        nc.vector.tensor_copy(Vs[:, t32 * OC + o * 128: t32 * OC + (o + 1) * 128], vtp[:])

        # ---- attention constants (emitted after proj so the first xt DMAs lead)
        maskT = persist.tile([128, 128], F32, name="maskT")
        nc.gpsimd.memset(maskT[:], 0.0)
        # keep 0 where q >= kk (predicate -x + y >= 0), else MASK_VAL
        nc.gpsimd.affine_select(
            out=maskT[:], in_=maskT[:], compare_op=mybir.AluOpType.is_ge,
            fill=MASK_VAL, base=0, pattern=[[1, 128]], channel_multiplier=-1,
        )
        onesk = persist.tile([128, 128], MDT, name="onesk")
        zeros = persist.tile([128, 128], MDT, name="zeros")
        with tc.tile_pool(name="cstage", bufs=1) as cstage:
            ones_f = cstage.tile([128, 128], F32, name="ones_f")
            nc.gpsimd.memset(ones_f[:], 1.0)
            nc.vector.tensor_copy(onesk[:], ones_f[:])
            zeros_f = cstage.tile([128, 128], F32, name="zeros_f")
            nc.gpsimd.memset(zeros_f[:], 0.0)
            nc.vector.tensor_copy(zeros[:], zeros_f[:])


        # =================== Phase 2: attention ===================
        a2a_in = [dram.tile([NCORES, 128, TCHUNK], MDT, name=f"a2a_in{h}") for h in range(HL)]
        a2a_out = [dram.tile([NCORES, 128, TCHUNK], MDT, name=f"a2a_out{h}")
                   for h in range(HL)]

        # wo-phase pools opened BEFORE the attention pool so woT prefetch and
        # the hl=0 ctx loads can run concurrently with attention compute.
        wope = ctx.enter_context(tc.tile_pool(name="wope", bufs=1))
        wotp = ctx.enter_context(tc.tile_pool(name="wotp", bufs=4))
        cm = [None] * (2 * NCORES)
        c_order = [2 * i for i in range(NCORES)] + [2 * i + 1 for i in range(NCORES)]
        wts = {}

        with tc.tile_pool(name="ptp", bufs=2) as ptp:
            for hl in range(HL):
                for J in reversed(range(NQC)):
                    for b in range(B):
                        base = hl * TT + b * T
                        nkk = 4 * J + 4
                        ptiles = []
                        for kk in range(nkk):
                            pt = ptp.tile([128, 512], MDT, tag=f"pt{kk}", name=f"p_{hl}{b}{J}_{kk}", bufs=2 if kk < 8 else 1)
                            ptiles.append(pt)
                            s_off = max(0, (kk - 4 * J) * 128)
                            npr = 512 - s_off
                            st = psum.tile([128, 512], F32, tag="mm", name=f"st{hl}{b}{J}_{kk}")
                            nc.tensor.matmul(
                                st[:, :npr],
                                KT[:, base + kk * 128: base + (kk + 1) * 128],
                                QT[:, base + J * 512 + s_off: base + (J + 1) * 512],
                                start=True, stop=True,
                            )
                            if kk >= 4 * J:  # diagonal tile: causal mask
                                nc.vector.tensor_add(st[:, 0:128], st[:, 0:128], maskT[:])
                            nc.scalar.activation(pt[:, s_off:512], st[:, :npr],
                                                 mybir.ActivationFunctionType.Exp, scale=SCALE)
                            for zoff in range(0, s_off, 128):
                                nc.vector.tensor_copy(pt[:, zoff:zoff + 128], zeros[:])
                        # denominator (broadcast to all 128 partitions): d[p, q] = sum_kk P^T
                        dp = psum.tile([128, 512], F32, tag="mm", name=f"dp{hl}{b}{J}")
                        for kk in range(nkk):
                            nc.tensor.matmul(dp[:], onesk[:], ptiles[kk][:],
                                             start=(kk == 0), stop=(kk == nkk - 1))
                        dsb = small.tile([128, 512], F32, tag="dsb", name=f"dsb_{hl}{b}{J}")
                        nc.scalar.copy(dsb[:], dp[:])
                        rd = small.tile([128, 512], F32, tag="rd", name=f"rd_{hl}{b}{J}")
                        nc.vector.reciprocal_approx_fast(rd[:], dsb[:])
                        # ctx^T accumulate over kk
                        cp = psum.tile([128, 512], F32, tag="mm", name=f"cp{hl}{b}{J}")
                        for kk in range(nkk):
                            nc.tensor.matmul(
                                cp[:],
                                Vs[:, (b * 16 + kk) * OC + hl * 128: (b * 16 + kk) * OC + (hl + 1) * 128],
                                ptiles[kk][:],
                                start=(kk == 0), stop=(kk == nkk - 1),
                            )
                        csb = small.tile([128, 512], MDT, tag="csb", name=f"csb{hl}{b}{J}", bufs=3)
                        nc.vector.tensor_mul(csb[:], cp[:], rd[:])
                        nc.sync.dma_start(out=a2a_in[hl][b * NQC + J], in_=csb[:])
                nc.gpsimd.collective_compute(
                    "AllToAll", mybir.AluOpType.bypass,
                    replica_groups=[list(range(NCORES))],
                    ins=[a2a_in[hl].opt()], outs=[a2a_out[hl].opt()],
                )
            # after both heads' attention: load hl=0 ctx slices on the scalar
            # (ACT) queue -- its exp backlog is done, and A2A0 finished long ago
            cme = wope.tile([128, NCORES * TCHUNK], MDT, name="cme")
            nc.gpsimd.dma_start(
                out=cme[:].rearrange("p (c t) -> p c t", c=NCORES),
                in_=a2a_out[0].rearrange("c p t -> p c t"),
            )

        # =================== Phase 3: output projection ===================
        # Evens (ready after A2A0) accumulate for ALL o4 groups while A2A1 is
        # in flight, spilling partials to SBUF; odds then accumulate into
        # fresh PSUM and a DVE add merges the halves on the way out.
        with tc.tile_pool(name="wopo", bufs=1) as wopo, \
             tc.tile_pool(name="accp", bufs=1) as accp, \
             tc.tile_pool(name="outp", bufs=3) as outp:
            cmo = wopo.tile([128, NCORES * TCHUNK], MDT, name="cmo")
            nc.gpsimd.dma_start(
                out=cmo[:].rearrange("p (c t) -> p c t", c=NCORES),
                in_=a2a_out[1].rearrange("c p t -> p c t"),
            )
            evens = c_order[:NCORES]
            odds = c_order[NCORES:]
            acc = {}
            for pi, (oa, ob) in enumerate(((0, 1), (2, 3))):
                for ci, c16 in enumerate(evens):
                    wt = wotp.tile([128, 1024], MDT, tag="wot", name=f"wte{pi}_{c16}")
                    eng = (nc.scalar, nc.sync)[ci % 2]
                    eng.dma_start(
                        out=wt[:],
                        in_=wot_d.ap()[c16 * 128:(c16 + 1) * 128, oa * 512:(ob + 1) * 512],
                    )
                    wts[(pi, c16)] = wt
                ops = {o4: [psum.tile([128, 512], F32, tag="mm", name=f"ope{o4}_{t}") for t in range(4)]
                       for o4 in (oa, ob)}
                for ci, c16 in enumerate(evens):
                    wt = wts[(pi, c16)]
                    i = c16 // 2
                    for oi, o4 in enumerate((oa, ob)):
                        for t4 in range(4):
                            nc.tensor.matmul(ops[o4][t4][:],
                                             cme[:, i * 512 + t4 * 128: i * 512 + (t4 + 1) * 128],
                                             wt[:, oi * 512:(oi + 1) * 512],
                                             start=(ci == 0), stop=(ci == NCORES - 1))
                for o4 in (oa, ob):
                    for t4 in range(4):
                        a_ = accp.tile([128, 512], F32, name=f"acc{o4}_{t4}")
                        nc.scalar.copy(a_[:], ops[o4][t4][:])
                        acc[(o4, t4)] = a_
            for pi, (oa, ob) in enumerate(((0, 1), (2, 3))):
                for ci, c16 in enumerate(odds):
                    wt = wotp.tile([128, 1024], MDT, tag="wot", name=f"wto{pi}_{c16}")
                    eng = (nc.scalar, nc.sync, nc.gpsimd)[ci % 3]
                    eng.dma_start(
                        out=wt[:],
                        in_=wot_d.ap()[c16 * 128:(c16 + 1) * 128, oa * 512:(ob + 1) * 512],
                    )
                    wts[("o", pi, c16)] = wt
                ops = {o4: [psum.tile([128, 512], F32, tag="mm", name=f"opo{o4}_{t}") for t in range(4)]
                       for o4 in (oa, ob)}
                for ci, c16 in enumerate(odds):
                    wt = wts[("o", pi, c16)]
                    i = c16 // 2
                    for oi, o4 in enumerate((oa, ob)):
                        for t4 in range(4):
                            nc.tensor.matmul(ops[o4][t4][:],
                                             cmo[:, i * 512 + t4 * 128: i * 512 + (t4 + 1) * 128],
                                             wt[:, oi * 512:(oi + 1) * 512],
                                             start=(ci == 0), stop=(ci == NCORES - 1))
                for o4 in (oa, ob):
                    for t4 in range(4):
                        ot = outp.tile([128, 512], F32, tag="ot", name=f"ot{o4}_{t4}")
                        nc.vector.tensor_add(ot[:], ops[o4][t4][:], acc[(o4, t4)][:])
                        nc.sync.dma_start(
                            out=out_d.ap()[t4 * 128:(t4 + 1) * 128, o4 * 512:(o4 + 1) * 512],
                            in_=ot[:],
                        )

    nc.compile()
    return nc


def get_nc():
    if "nc" not in _CACHE:
        _CACHE["nc"] = build()
    return _CACHE["nc"]


def make_in_maps(x, wq, wk, wv, wo):
    x = np.asarray(x, dtype=np.float32)
    xT = np.ascontiguousarray(x.reshape(TT, D).T)
    woT = np.ascontiguousarray(np.asarray(wo, np.float32).T)
    in_maps = []
    for i in range(NCORES):
        sl = slice(i * OC, (i + 1) * OC)
        in_maps.append({
            "xt": xT,
            "wqt": np.ascontiguousarray(np.asarray(wq, np.float32)[sl, :].T),
            "wkt": np.ascontiguousarray(np.asarray(wk, np.float32)[sl, :].T),
            "wvt": np.ascontiguousarray(np.asarray(wv, np.float32)[sl, :].T),
            "wot": woT,
        })
    return in_maps


def assemble(results):
    return np.concatenate([results[i]["out"] for i in range(NCORES)], axis=0).reshape(B, T, D)


def kernel(x, wq, wk, wv, wo):
    nc = get_nc()
    in_maps = make_in_maps(x, wq, wk, wv, wo)
    res = run_bass_kernel_spmd(nc, in_maps, list(range(NCORES)), trace=False)
    return assemble(res.results)


if __name__ == "__main__":
    rng = np.random.default_rng(0)
    s = 1.0 / math.sqrt(D)
    x = rng.standard_normal((B, T, D), dtype=np.float32)
    wq = (rng.standard_normal((D, D), dtype=np.float32) * s)
    wk = (rng.standard_normal((D, D), dtype=np.float32) * s)
    wv = (rng.standard_normal((D, D), dtype=np.float32) * s)
    wo = (rng.standard_normal((D, D), dtype=np.float32) * s)
    out = kernel(x, wq, wk, wv, wo)
    print("out", out.shape, out.dtype, np.abs(out).mean())



# revision 5
# speedup vs baseline: 1.3025x; 1.3025x over previous
"""Multi-head causal attention (B=2, T=2048, D=2048, H=16, dk=128) on 8 TRN2 NeuronCores.

Strategy (tensor-parallel over heads, 2 heads/core), v2:
  - Everything matmul-facing is bf16 (host-converted): halves DMA traffic,
    SBUF footprint, LDWEIGHTS time and A2A payload at the same PE rate as
    fp32r. PSUM/denominators/output stay f32.
  - Host prep: transpose x -> xT [D, B*T], slice+transpose Wq/Wk/Wv per core
    ([D, 256] each), full Wo.T; all bf16.
  - Per core proj: Q^T/K^T [dk, tokens] via PE matmuls; V computed DIRECTLY in
    natural [tokens, dk] layout (x-tile stationary, wv moving) -- no PE
    transposes, no identity. x stream is split across two DMA queues
    (gpsimd: k 0..7, vector: k 8..15), 4 k-tiles per DMA issue. The full Wo
    (bf16, 8 MB) is prefetched into SBUF during proj, 4 tiles per tcx on sync.
  - Attention with TRANSPOSED scores S^T[kk, q] per kk-tile; exp'd probs land
    in P^T layout bf16. No max-subtraction (scores ~N(0,1)). Softmax
    denominator via ones[128,128] matmul over P^T partitions; partially-masked
    diagonal tiles only compute/accumulate their live q-columns (no zero
    fills). 1/d via reciprocal_approx_fast applied in the ctx PSUM->SBUF mul.
  - ctx^T bf16 -> AllToAll (1 MB/core per head) so each core gets all 16
    heads' ctx^T for its 512-token slice. Collective outputs in Shared DRAM.
    cme/cmo gathers are on sync AFTER all csb stores so they never block
    attention-critical work nor the second collective trigger (keeping both
    triggers adjacent on gpsimd).
  - Wo: evens (c-tiles from A2A0) accumulate during A2A1, spill to SBUF via
    ACT; odds accumulate in fresh PSUM; DVE add merges on the way out.
  - Host: concatenate the 8 row-slices.
"""

import math
import numpy as np
import ml_dtypes
from contextlib import ExitStack

import concourse.tile as tile
import concourse.mybir as mybir
from concourse import bacc
from concourse.bass_utils import run_bass_kernel_spmd

B, T, D = 2, 2048, 2048
H, DK = 16, 128
NCORES = 8
HL = H // NCORES            # 2 heads per core
OC = HL * DK                # 256 out dims per core
TT = B * T                  # 4096 flat tokens
TCHUNK = 512
NTC = TT // TCHUNK          # 8 token chunks (proj)
NKT = D // 128              # 16 contraction tiles
NQC = T // TCHUNK           # 4 q-chunks per batch
SCALE = 1.0 / math.sqrt(DK)
F32 = mybir.dt.float32
BF = mybir.dt.bfloat16
MASK_VAL = -1e30

_CACHE = {}


def build():
    nc = bacc.Bacc("TRN2", target_bir_lowering=False, debug=False, num_devices=NCORES)

    xt_d = nc.dram_tensor("xt", [D, TT], BF, kind="ExternalInput")
    wqt_d = nc.dram_tensor("wqt", [D, OC], BF, kind="ExternalInput")
    wkt_d = nc.dram_tensor("wkt", [D, OC], BF, kind="ExternalInput")
    wvt_d = nc.dram_tensor("wvt", [D, OC], BF, kind="ExternalInput")
    wot_d = nc.dram_tensor("wot", [D, D], BF, kind="ExternalInput")
    out_d = nc.dram_tensor("out", [TT // NCORES, D], F32, kind="ExternalOutput")

    with tile.TileContext(nc) as tc, ExitStack() as ctx:
        dram = ctx.enter_context(tc.tile_pool(name="dram", bufs=1, space="DRAM"))
        persist = ctx.enter_context(tc.tile_pool(name="persist", bufs=1))
        small = ctx.enter_context(tc.tile_pool(name="small", bufs=2))

        # ---- persistent SBUF: QT/KT [128, HL*TT] (cols: h*TT + flat_tok),
        # V natural [128, 32*256], full Wo (bf16), cme/cmo ctx gathers.
        QT = persist.tile([128, HL * TT], BF, name="QTs")
        KT = persist.tile([128, HL * TT], BF, name="KTs")
        Vs = persist.tile([128, (TT // 128) * OC], BF, name="Vs")
        wo_sb = persist.tile([128, 2 * D * 8], BF, name="wo_sb")  # 32 x [128,1024]
        cme = persist.tile([128, NCORES * TCHUNK], BF, name="cme")
        cmo = persist.tile([128, NCORES * TCHUNK], BF, name="cmo")

        # =================== Phase 1: QKV projections ===================
        with tc.tile_pool(name="proj", bufs=1) as projp, \
             tc.tile_pool(name="xtp", bufs=6) as xtp, \
             tc.tile_pool(name="pq", bufs=1, space="PSUM") as pqp, \
             tc.tile_pool(name="pv", bufs=1, space="PSUM") as pvp:
            wq_sb = projp.tile([128, NKT * OC], BF, name="wq_sb")
            wk_sb = projp.tile([128, NKT * OC], BF, name="wk_sb")
            wv_sb = projp.tile([128, NKT * OC], BF, name="wv_sb")
            # split weight loads into k-groups so the first matmuls start early
            for kg in range(0, NKT, 4):
                for w_sb, w_d in ((wq_sb, wqt_d), (wk_sb, wkt_d), (wv_sb, wvt_d)):
                    nc.sync.dma_start(
                        out=w_sb[:, kg * OC:(kg + 4) * OC].rearrange("p (kt o) -> p kt o", kt=4),
                        in_=w_d.ap()[kg * 128:(kg + 4) * 128, :].rearrange("(kt p) o -> p kt o", p=128),
                    )

            for tcx in range(NTC):
                # x stream: 4 k-tiles per DMA, split across gpsimd/vector queues
                xgs = []
                for g in range(4):
                    xg = xtp.tile([128, 4 * TCHUNK], BF, tag="xt", name=f"xg_{tcx}_{g}")
                    eng = nc.gpsimd if g < 2 else nc.scalar
                    eng.dma_start(
                        out=xg[:].rearrange("p (kt t) -> p kt t", kt=4),
                        in_=xt_d.ap()[g * 512:(g + 1) * 512, tcx * TCHUNK:(tcx + 1) * TCHUNK]
                            .rearrange("(kt p) t -> p kt t", p=128),
                    )
                    xgs.append(xg)
                # Wo prefetch: 4 tiles of [128, 1024] per tcx on sync
                for j in range(4):
                    widx = tcx * 4 + j
                    c16, pi = widx // 2, widx % 2
                    nc.sync.dma_start(
                        out=wo_sb[:, widx * 1024:(widx + 1) * 1024],
                        in_=wot_d.ap()[c16 * 128:(c16 + 1) * 128, pi * 1024:(pi + 1) * 1024],
                    )

                qp = [pqp.tile([128, 512], F32, tag=f"qp{o}", name=f"qp{tcx}_{o}") for o in range(HL)]
                kp = [pqp.tile([128, 512], F32, tag=f"kp{o}", name=f"kp{tcx}_{o}") for o in range(HL)]
                vp = [pvp.tile([128, 256], F32, tag=f"vp{t}", name=f"vp{tcx}_{t}") for t in range(4)]
                for k in range(NKT):
                    st_, sp_ = (k == 0), (k == NKT - 1)
                    g, kt = divmod(k, 4)
                    xsl = xgs[g][:, kt * TCHUNK:(kt + 1) * TCHUNK]
                    for o in range(HL):
                        nc.tensor.matmul(qp[o][:], wq_sb[:, k * OC + o * 128: k * OC + (o + 1) * 128],
                                         xsl, start=st_, stop=sp_)
                        nc.tensor.matmul(kp[o][:], wk_sb[:, k * OC + o * 128: k * OC + (o + 1) * 128],
                                         xsl, start=st_, stop=sp_)
                    for t4 in range(4):
                        nc.tensor.matmul(vp[t4][:], xsl[:, t4 * 128:(t4 + 1) * 128],
                                         wv_sb[:, k * OC:(k + 1) * OC], start=st_, stop=sp_)
                for o in range(HL):
                    dst = slice(o * TT + tcx * TCHUNK, o * TT + (tcx + 1) * TCHUNK)
                    nc.scalar.copy(QT[:, dst], qp[o][:])
                    nc.scalar.copy(KT[:, dst], kp[o][:])
                for t4 in range(4):
                    t32 = tcx * 4 + t4
                    nc.scalar.copy(Vs[:, t32 * OC:(t32 + 1) * OC], vp[t4][:])

        # ---- attention constants (emitted after proj so the first xt DMAs lead)
        maskT = persist.tile([128, 128], F32, name="maskT")
        nc.gpsimd.memset(maskT[:], 0.0)
        # keep 0 where q >= kk (predicate -x + y >= 0), else MASK_VAL
        nc.gpsimd.affine_select(
            out=maskT[:], in_=maskT[:], compare_op=mybir.AluOpType.is_ge,
            fill=MASK_VAL, base=0, pattern=[[1, 128]], channel_multiplier=-1,
        )
        onesk = persist.tile([128, 128], BF, name="onesk")
        with tc.tile_pool(name="cstage", bufs=1) as cstage:
            ones_f = cstage.tile([128, 128], F32, name="ones_f")
            nc.gpsimd.memset(ones_f[:], 1.0)
            nc.vector.tensor_copy(onesk[:], ones_f[:])

        # =================== Phase 2: attention ===================
        a2a_in = [dram.tile([NCORES, 128, TCHUNK], BF, name=f"a2a_in{h}") for h in range(HL)]
        a2a_out = [dram.tile([NCORES, 128, TCHUNK], BF, name=f"a2a_out{h}")
                   for h in range(HL)]

        with tc.tile_pool(name="ptp", bufs=2) as ptp, \
             tc.tile_pool(name="psA", bufs=8, space="PSUM") as psum:
            for hl in range(HL):
                for J in reversed(range(NQC)):
                    for b in range(B):
                        base = hl * TT + b * T
                        nkk = 4 * J + 4
                        ptiles = []
                        soffs = []
                        for kk in range(nkk):
                            pt = ptp.tile([128, 512], BF, tag=f"pt{kk}", name=f"p_{hl}{b}{J}_{kk}", bufs=2 if kk < 8 else 1)
                            ptiles.append(pt)
                            s_off = max(0, (kk - 4 * J) * 128)
                            soffs.append(s_off)
                            npr = 512 - s_off
                            st = psum.tile([128, 512], F32, tag="mm", name=f"st{hl}{b}{J}_{kk}")
                            nc.tensor.matmul(
                                st[:, :npr],
                                KT[:, base + kk * 128: base + (kk + 1) * 128],
                                QT[:, base + J * 512 + s_off: base + (J + 1) * 512],
                                start=True, stop=True,
                            )
                            if kk >= 4 * J:  # diagonal tile: causal mask
                                nc.vector.tensor_add(st[:, 0:128], st[:, 0:128], maskT[:])
                            nc.scalar.activation(pt[:, s_off:512], st[:, :npr],
                                                 mybir.ActivationFunctionType.Exp, scale=SCALE)
                        # denominator (broadcast to all 128 partitions): d[p, q] = sum_kk P^T
                        dp = psum.tile([128, 512], F32, tag="mm", name=f"dp{hl}{b}{J}")
                        for kk in range(nkk):
                            so = soffs[kk]
                            nc.tensor.matmul(dp[:, so:512], onesk[:], ptiles[kk][:, so:512],
                                             start=(kk == 0), stop=(kk == nkk - 1),
                                             skip_group_check=True)
                        dsb = small.tile([128, 512], F32, tag="dsb", name=f"dsb_{hl}{b}{J}")
                        nc.scalar.copy(dsb[:], dp[:])
                        rd = small.tile([128, 512], F32, tag="rd", name=f"rd_{hl}{b}{J}")
                        nc.vector.reciprocal_approx_fast(rd[:], dsb[:])
                        # ctx^T accumulate over kk
                        cp = psum.tile([128, 512], F32, tag="mm", name=f"cp{hl}{b}{J}")
                        for kk in range(nkk):
                            so = soffs[kk]
                            nc.tensor.matmul(
                                cp[:, so:512],
                                Vs[:, (b * 16 + kk) * OC + hl * 128: (b * 16 + kk) * OC + (hl + 1) * 128],
                                ptiles[kk][:, so:512],
                                start=(kk == 0), stop=(kk == nkk - 1),
                                skip_group_check=True,
                            )
                        csb = small.tile([128, 512], BF, tag="csb", name=f"csb{hl}{b}{J}", bufs=3)
                        nc.vector.tensor_mul(csb[:], cp[:], rd[:])
                        nc.sync.dma_start(out=a2a_in[hl][b * NQC + J], in_=csb[:])
                nc.gpsimd.collective_compute(
                    "AllToAll", mybir.AluOpType.bypass,
                    replica_groups=[list(range(NCORES))],
                    ins=[a2a_in[hl].opt()], outs=[a2a_out[hl].opt()],
                )
            # ctx gathers AFTER all csb stores (sync): never blocks attention
            # work; waits only on the matching collective's semaphore.
            nc.sync.dma_start(
                out=cme[:].rearrange("p (c t) -> p c t", c=NCORES),
                in_=a2a_out[0].rearrange("c p t -> p c t"),
            )
            nc.sync.dma_start(
                out=cmo[:].rearrange("p (c t) -> p c t", c=NCORES),
                in_=a2a_out[1].rearrange("c p t -> p c t"),
            )

        # =================== Phase 3: output projection ===================
        # Evens (ready after A2A0) accumulate for ALL o4 groups while A2A1 is
        # in flight, spilling partials to SBUF; odds then accumulate into
        # fresh PSUM and a DVE add merges the halves on the way out.
        with tc.tile_pool(name="accp", bufs=1) as accp, \
             tc.tile_pool(name="outp", bufs=3) as outp, \
             tc.tile_pool(name="psW", bufs=8, space="PSUM") as psw:
            acc = {}
            for pi, (oa, ob) in enumerate(((0, 1), (2, 3))):
                ops = {o4: [psw.tile([128, 512], F32, tag="mm", name=f"ope{o4}_{t}") for t in range(4)]
                       for o4 in (oa, ob)}
                for ci in range(NCORES):
                    c16 = 2 * ci
                    for oi, o4 in enumerate((oa, ob)):
                        for t4 in range(4):
                            nc.tensor.matmul(ops[o4][t4][:],
                                             cme[:, ci * 512 + t4 * 128: ci * 512 + (t4 + 1) * 128],
                                             wo_sb[:, (c16 * 2 + pi) * 1024 + oi * 512:
                                                   (c16 * 2 + pi) * 1024 + (oi + 1) * 512],
                                             start=(ci == 0), stop=(ci == NCORES - 1))
                for o4 in (oa, ob):
                    for t4 in range(4):
                        a_ = accp.tile([128, 512], F32, name=f"acc{o4}_{t4}")
                        nc.scalar.copy(a_[:], ops[o4][t4][:])
                        acc[(o4, t4)] = a_
            for pi, (oa, ob) in enumerate(((0, 1), (2, 3))):
                ops = {o4: [psw.tile([128, 512], F32, tag="mm", name=f"opo{o4}_{t}") for t in range(4)]
                       for o4 in (oa, ob)}
                for ci in range(NCORES):
                    c16 = 2 * ci + 1
                    for oi, o4 in enumerate((oa, ob)):
                        for t4 in range(4):
                            nc.tensor.matmul(ops[o4][t4][:],
                                             cmo[:, ci * 512 + t4 * 128: ci * 512 + (t4 + 1) * 128],
                                             wo_sb[:, (c16 * 2 + pi) * 1024 + oi * 512:
                                                   (c16 * 2 + pi) * 1024 + (oi + 1) * 512],
                                             start=(ci == 0), stop=(ci == NCORES - 1))
                for o4 in (oa, ob):
                    for t4 in range(4):
                        ot = outp.tile([128, 512], F32, tag="ot", name=f"ot{o4}_{t4}")
                        nc.vector.tensor_add(ot[:], ops[o4][t4][:], acc[(o4, t4)][:])
                        nc.sync.dma_start(
                            out=out_d.ap()[t4 * 128:(t4 + 1) * 128, o4 * 512:(o4 + 1) * 512],
                            in_=ot[:],
                        )

    nc.compile()
    return nc


def get_nc():
    if "nc" not in _CACHE:
        _CACHE["nc"] = build()
    return _CACHE["nc"]


def make_in_maps(x, wq, wk, wv, wo):
    bf = ml_dtypes.bfloat16
    x = np.asarray(x, dtype=np.float32)
    xT = np.ascontiguousarray(x.reshape(TT, D).T.astype(bf))
    woT = np.ascontiguousarray(np.asarray(wo, np.float32).T.astype(bf))
    in_maps = []
    for i in range(NCORES):
        sl = slice(i * OC, (i + 1) * OC)
        in_maps.append({
            "xt": xT,
            "wqt": np.ascontiguousarray(np.asarray(wq, np.float32)[sl, :].T.astype(bf)),
            "wkt": np.ascontiguousarray(np.asarray(wk, np.float32)[sl, :].T.astype(bf)),
            "wvt": np.ascontiguousarray(np.asarray(wv, np.float32)[sl, :].T.astype(bf)),
            "wot": woT,
        })
    return in_maps


def assemble(results):
    return np.concatenate([results[i]["out"] for i in range(NCORES)], axis=0).reshape(B, T, D)


def kernel(x, wq, wk, wv, wo):
    nc = get_nc()
    in_maps = make_in_maps(x, wq, wk, wv, wo)
    res = run_bass_kernel_spmd(nc, in_maps, list(range(NCORES)), trace=False)
    return assemble(res.results)


if __name__ == "__main__":
    rng = np.random.default_rng(0)
    s = 1.0 / math.sqrt(D)
    x = rng.standard_normal((B, T, D), dtype=np.float32)
    wq = (rng.standard_normal((D, D), dtype=np.float32) * s)
    wk = (rng.standard_normal((D, D), dtype=np.float32) * s)
    wv = (rng.standard_normal((D, D), dtype=np.float32) * s)
    wo = (rng.standard_normal((D, D), dtype=np.float32) * s)
    out = kernel(x, wq, wk, wv, wo)
    print("out", out.shape, out.dtype, np.abs(out).mean())


# revision 7
# speedup vs baseline: 1.3622x; 1.0459x over previous
"""Multi-head causal attention (B=2, T=2048, D=2048, H=16, dk=128) on 8 TRN2 NeuronCores.

Strategy (tensor-parallel over heads, 2 heads/core), v3:
  - Everything matmul-facing is bf16 (host-converted): halves DMA traffic,
    SBUF footprint, LDWEIGHTS time and A2A payload at the same PE rate.
    PSUM/denominators/output stay f32.
  - Host prep: transpose x -> xT [D, B*T], slice+transpose Wq/Wk/Wv per core
    ([D, 256] each), full Wo.T; all bf16.
  - Per core proj: Q^T/K^T/V^T [dk, tokens] via N=512 PE matmuls; V^T is
    turned into natural-layout V via the DMA-transpose XBAR (no PE
    transposes). x stream split across two DMA queues (gpsimd: k 0..7,
    scalar: k 8..15), 4 k-tiles per issue. Full Wo (bf16, 8 MB) prefetched
    into SBUF during proj, 4 tiles per tcx on sync.
  - Attention with TRANSPOSED scores S^T[kk, q] per kk-tile; exp'd probs land
    in P^T layout bf16. No max-subtraction (scores ~N(0,1)). Softmax
    denominator via ones[128,128] matmul over P^T partitions; partially-masked
    diagonal tiles only compute/accumulate live q-columns. 1/d via DVE
    reciprocal_approx_fast straight from PSUM, applied in the ctx mul.
    The PE stream is software-pipelined one group deep: scores of group g+1
    are emitted before the denominator/ctx chains of group g, so those chains
    never wait on freshly-exp'd tiles.
  - ctx^T bf16 -> AllToAll (1 MB/core per head). cme/cmo gathers on sync
    AFTER all csb stores; collective triggers adjacent on gpsimd.
  - Wo: tile-major accumulation (each output tile finishes early and its
    spill/merge/store overlaps the next tile). Evens (A2A0 c-tiles) run
    during A2A1; odds accumulate in fresh PSUM; DVE add merges; stores
    alternate sync/scalar.
  - Host: concatenate the 8 row-slices.
"""

import math
import numpy as np
import ml_dtypes
from contextlib import ExitStack

import concourse.tile as tile
import concourse.mybir as mybir
from concourse import bacc
from concourse.bass_utils import run_bass_kernel_spmd

B, T, D = 2, 2048, 2048
H, DK = 16, 128
NCORES = 8
HL = H // NCORES            # 2 heads per core
OC = HL * DK                # 256 out dims per core
TT = B * T                  # 4096 flat tokens
TCHUNK = 512
NTC = TT // TCHUNK          # 8 token chunks (proj)
NKT = D // 128              # 16 contraction tiles
NQC = T // TCHUNK           # 4 q-chunks per batch
SCALE = 1.0 / math.sqrt(DK)
F32 = mybir.dt.float32
BF = mybir.dt.bfloat16
MASK_VAL = -1e30

_CACHE = {}


def build():
    nc = bacc.Bacc("TRN2", target_bir_lowering=False, debug=False, num_devices=NCORES)

    xt_d = nc.dram_tensor("xt", [D, TT], BF, kind="ExternalInput")
    wqt_d = nc.dram_tensor("wqt", [D, OC], BF, kind="ExternalInput")
    wkt_d = nc.dram_tensor("wkt", [D, OC], BF, kind="ExternalInput")
    wvt_d = nc.dram_tensor("wvt", [D, OC], BF, kind="ExternalInput")
    wot_d = nc.dram_tensor("wot", [D, D], BF, kind="ExternalInput")
    out_d = nc.dram_tensor("out", [TT // NCORES, D], F32, kind="ExternalOutput")

    with tile.TileContext(nc) as tc, ExitStack() as ctx:
        dram = ctx.enter_context(tc.tile_pool(name="dram", bufs=1, space="DRAM"))
        persist = ctx.enter_context(tc.tile_pool(name="persist", bufs=1))
        small = ctx.enter_context(tc.tile_pool(name="small", bufs=2))

        QT = persist.tile([128, HL * TT], BF, name="QTs")
        KT = persist.tile([128, HL * TT], BF, name="KTs")
        Vs = persist.tile([128, (TT // 128) * OC], BF, name="Vs")
        wo_sb = persist.tile([128, 2 * D * 8], BF, name="wo_sb")  # 32 x [128,1024]
        cme = persist.tile([128, NCORES * TCHUNK], BF, name="cme")
        cmo = persist.tile([128, NCORES * TCHUNK], BF, name="cmo")

        # ---- attention constants, staged while the first DMAs are in flight
        maskT = persist.tile([128, 128], F32, name="maskT")
        nc.gpsimd.memset(maskT[:], 0.0)
        # keep 0 where q >= kk (predicate -x + y >= 0), else MASK_VAL
        nc.gpsimd.affine_select(
            out=maskT[:], in_=maskT[:], compare_op=mybir.AluOpType.is_ge,
            fill=MASK_VAL, base=0, pattern=[[1, 128]], channel_multiplier=-1,
        )
        onesk = persist.tile([128, 128], BF, name="onesk")
        with tc.tile_pool(name="cstage", bufs=1) as cstage:
            ones_f = cstage.tile([128, 128], F32, name="ones_f")
            nc.gpsimd.memset(ones_f[:], 1.0)
            nc.vector.tensor_copy(onesk[:], ones_f[:])

        # =================== Phase 1: QKV projections ===================
        with tc.tile_pool(name="proj", bufs=1) as projp, \
             tc.tile_pool(name="xtp", bufs=6) as xtp, \
             tc.tile_pool(name="vtsp", bufs=2) as vtsp, \
             tc.tile_pool(name="pq", bufs=1, space="PSUM") as pqp:
            wq_sb = projp.tile([128, NKT * OC], BF, name="wq_sb")
            wk_sb = projp.tile([128, NKT * OC], BF, name="wk_sb")
            wv_sb = projp.tile([128, NKT * OC], BF, name="wv_sb")
            # first k-groups are small so the first matmuls start early
            for ka, kb in ((0, 2), (2, 4), (4, 8), (8, 12), (12, 16)):
                for w_sb, w_d in ((wq_sb, wqt_d), (wk_sb, wkt_d), (wv_sb, wvt_d)):
                    nc.sync.dma_start(
                        out=w_sb[:, ka * OC:kb * OC].rearrange("p (kt o) -> p kt o", kt=kb - ka),
                        in_=w_d.ap()[ka * 128:kb * 128, :].rearrange("(kt p) o -> p kt o", p=128),
                    )

            for tcx in range(NTC):
                # x stream: 4 k-tiles per DMA, split across gpsimd/scalar queues
                xgs = []
                for g in range(4):
                    xg = xtp.tile([128, 4 * TCHUNK], BF, tag="xt", name=f"xg_{tcx}_{g}")
                    eng = nc.gpsimd if g < 2 else nc.scalar
                    eng.dma_start(
                        out=xg[:].rearrange("p (kt t) -> p kt t", kt=4),
                        in_=xt_d.ap()[g * 512:(g + 1) * 512, tcx * TCHUNK:(tcx + 1) * TCHUNK]
                            .rearrange("(kt p) t -> p kt t", p=128),
                    )
                    xgs.append(xg)
                # Wo prefetch: 4 tiles of [128, 1024] per tcx on sync
                for j in range(4):
                    widx = tcx * 4 + j
                    c16, pi = widx // 2, widx % 2
                    nc.sync.dma_start(
                        out=wo_sb[:, widx * 1024:(widx + 1) * 1024],
                        in_=wot_d.ap()[c16 * 128:(c16 + 1) * 128, pi * 1024:(pi + 1) * 1024],
                    )

                qp = [pqp.tile([128, 512], F32, tag=f"qp{o}", name=f"qp{tcx}_{o}", bufs=2)
                      for o in range(HL)]
                kp = [pqp.tile([128, 512], F32, tag=f"kp{o}", name=f"kp{tcx}_{o}") for o in range(HL)]
                vp = [pqp.tile([128, 512], F32, tag=f"vp{o}", name=f"vp{tcx}_{o}") for o in range(HL)]
                for k in range(NKT):
                    st_, sp_ = (k == 0), (k == NKT - 1)
                    g, kt = divmod(k, 4)
                    xsl = xgs[g][:, kt * TCHUNK:(kt + 1) * TCHUNK]
                    for o in range(HL):
                        nc.tensor.matmul(qp[o][:], wq_sb[:, k * OC + o * 128: k * OC + (o + 1) * 128],
                                         xsl, start=st_, stop=sp_)
                        nc.tensor.matmul(kp[o][:], wk_sb[:, k * OC + o * 128: k * OC + (o + 1) * 128],
                                         xsl, start=st_, stop=sp_)
                        nc.tensor.matmul(vp[o][:], wv_sb[:, k * OC + o * 128: k * OC + (o + 1) * 128],
                                         xsl, start=st_, stop=sp_)
                for o in range(HL):
                    dst = slice(o * TT + tcx * TCHUNK, o * TT + (tcx + 1) * TCHUNK)
                    nc.scalar.copy(QT[:, dst], qp[o][:])
                    nc.scalar.copy(KT[:, dst], kp[o][:])
                # V^T -> V via the DMA-transpose XBAR (per 128x128 block)
                for o in range(HL):
                    vts = vtsp.tile([128, 512], BF, tag="vts", name=f"vts{tcx}_{o}")
                    nc.scalar.copy(vts[:], vp[o][:])
                    eng = (nc.scalar, nc.sync)[o]
                    for t4 in range(4):
                        t32 = tcx * 4 + t4
                        eng.dma_start_transpose(
                            out=Vs[:, t32 * OC + o * 128: t32 * OC + (o + 1) * 128],
                            in_=vts[:, t4 * 128:(t4 + 1) * 128],
                        )

        # =================== Phase 2: attention ===================
        a2a_in = [dram.tile([NCORES, 128, TCHUNK], BF, name=f"a2a_in{h}") for h in range(HL)]
        a2a_out = [dram.tile([NCORES, 128, TCHUNK], BF, name=f"a2a_out{h}")
                   for h in range(HL)]

        with tc.tile_pool(name="ptp", bufs=2) as ptp, \
             tc.tile_pool(name="psA", bufs=1, space="PSUM") as psum:

            def start_group(hl, J, b):
                nkk = 4 * J + 4
                ptiles = [ptp.tile([128, 512], BF, tag=f"pt{kk}",
                                   name=f"p_{hl}{b}{J}_{kk}", bufs=2)
                          for kk in range(nkk)]
                soffs = [max(0, (kk - 4 * J) * 128) for kk in range(nkk)]
                dp = psum.tile([128, 512], F32, tag="dpx", name=f"dp{hl}{b}{J}", bufs=2)
                cp = psum.tile([128, 512], F32, tag="cpx", name=f"cp{hl}{b}{J}", bufs=2)
                return dict(hl=hl, J=J, b=b, nkk=nkk, ptiles=ptiles, soffs=soffs,
                            dp=dp, cp=cp)

            def emit_sc(g, kk):
                hl, J, b = g["hl"], g["J"], g["b"]
                base = hl * TT + b * T
                s_off = g["soffs"][kk]
                npr = 512 - s_off
                pt = g["ptiles"][kk]
                st = psum.tile([128, 512], F32, tag="st", name=f"st{hl}{b}{J}_{kk}", bufs=4)
                nc.tensor.matmul(
                    st[:, :npr],
                    KT[:, base + kk * 128: base + (kk + 1) * 128],
                    QT[:, base + J * 512 + s_off: base + (J + 1) * 512],
                    start=True, stop=True,
                )
                if kk >= 4 * J:  # diagonal tile: causal mask
                    nc.vector.tensor_add(st[:, 0:128], st[:, 0:128], maskT[:])
                nc.scalar.activation(pt[:, s_off:512], st[:, :npr],
                                     mybir.ActivationFunctionType.Exp, scale=SCALE)

            def emit_dpcp_tile(g, kk):
                so = g["soffs"][kk]
                pt = g["ptiles"][kk]
                st_, sp_ = (kk == 0), (kk == g["nkk"] - 1)
                nc.tensor.matmul(g["dp"][:, so:512], onesk[:], pt[:, so:512],
                                 start=st_, stop=sp_, skip_group_check=True)
                b, hl = g["b"], g["hl"]
                nc.tensor.matmul(
                    g["cp"][:, so:512],
                    Vs[:, (b * 16 + kk) * OC + hl * 128: (b * 16 + kk) * OC + (hl + 1) * 128],
                    pt[:, so:512],
                    start=st_, stop=sp_, skip_group_check=True,
                )

            def finish_group(g):
                hl, J, b = g["hl"], g["J"], g["b"]
                rd = small.tile([128, 512], F32, tag="rd", name=f"rd_{hl}{b}{J}")
                nc.vector.reciprocal_approx_fast(rd[:], g["dp"][:])
                csb = small.tile([128, 512], BF, tag="csb", name=f"csb{hl}{b}{J}", bufs=3)
                nc.vector.tensor_mul(csb[:], g["cp"][:], rd[:])
                nc.sync.dma_start(out=a2a_in[hl][b * NQC + J], in_=csb[:])
                if (J, b) == (0, B - 1):
                    nc.gpsimd.collective_compute(
                        "AllToAll", mybir.AluOpType.bypass,
                        replica_groups=[list(range(NCORES))],
                        ins=[a2a_in[hl].opt()], outs=[a2a_out[hl].opt()],
                    )

            # One-group-deep software pipeline with per-kk interleave: the PE
            # stream alternates sc(g+1, kk) with dp/cp(g, kk), so neither the
            # exp latency of the current group nor PSUM rotation ever stalls it.
            groups = [(hl, J, b) for hl in range(HL)
                      for J in reversed(range(NQC)) for b in range(B)]
            prev = None
            for (hl, J, b) in groups:
                cur = start_group(hl, J, b)
                for kk in range(max(cur["nkk"], prev["nkk"] if prev else 0)):
                    if kk < cur["nkk"]:
                        emit_sc(cur, kk)
                    if prev is not None and kk < prev["nkk"]:
                        emit_dpcp_tile(prev, kk)
                if prev is not None:
                    finish_group(prev)
                prev = cur
            for kk in range(prev["nkk"]):
                emit_dpcp_tile(prev, kk)
            finish_group(prev)
            # ctx gathers AFTER all csb stores (sync): never blocks attention
            # work; waits only on the matching collective's semaphore.
            nc.sync.dma_start(
                out=cme[:].rearrange("p (c t) -> p c t", c=NCORES),
                in_=a2a_out[0].rearrange("c p t -> p c t"),
            )
            nc.sync.dma_start(
                out=cmo[:].rearrange("p (c t) -> p c t", c=NCORES),
                in_=a2a_out[1].rearrange("c p t -> p c t"),
            )

        # =================== Phase 3: output projection ===================
        # Tile-major: each output tile accumulates its 8 c-tiles back-to-back
        # and is spilled (evens) or merged+stored (odds) while the next tile
        # accumulates. Evens run during A2A1.
        with tc.tile_pool(name="accp", bufs=1) as accp, \
             tc.tile_pool(name="outp", bufs=3) as outp, \
             tc.tile_pool(name="psW", bufs=8, space="PSUM") as psw:
            acc = {}
            for pi, (oa, ob) in enumerate(((0, 1), (2, 3))):
                for oi, o4 in enumerate((oa, ob)):
                    for t4 in range(4):
                        pe = psw.tile([128, 512], F32, tag="mm", name=f"ope{o4}_{t4}")
                        for ci in range(NCORES):
                            c16 = 2 * ci
                            nc.tensor.matmul(pe[:],
                                             cme[:, ci * 512 + t4 * 128: ci * 512 + (t4 + 1) * 128],
                                             wo_sb[:, (c16 * 2 + pi) * 1024 + oi * 512:
                                                   (c16 * 2 + pi) * 1024 + (oi + 1) * 512],
                                             start=(ci == 0), stop=(ci == NCORES - 1))
                        a_ = accp.tile([128, 512], F32, name=f"acc{o4}_{t4}")
                        nc.scalar.copy(a_[:], pe[:])
                        acc[(o4, t4)] = a_
            si = 0
            for pi, (oa, ob) in enumerate(((0, 1), (2, 3))):
                for oi, o4 in enumerate((oa, ob)):
                    for t4 in range(4):
                        po = psw.tile([128, 512], F32, tag="mm", name=f"opo{o4}_{t4}")
                        for ci in range(NCORES):
                            c16 = 2 * ci + 1
                            nc.tensor.matmul(po[:],
                                             cmo[:, ci * 512 + t4 * 128: ci * 512 + (t4 + 1) * 128],
                                             wo_sb[:, (c16 * 2 + pi) * 1024 + oi * 512:
                                                   (c16 * 2 + pi) * 1024 + (oi + 1) * 512],
                                             start=(ci == 0), stop=(ci == NCORES - 1))
                        ot = outp.tile([128, 512], F32, tag="ot", name=f"ot{o4}_{t4}")
                        nc.vector.tensor_add(ot[:], po[:], acc[(o4, t4)][:])
                        eng = (nc.sync, nc.scalar)[si % 2]
                        si += 1
                        eng.dma_start(
                            out=out_d.ap()[t4 * 128:(t4 + 1) * 128, o4 * 512:(o4 + 1) * 512],
                            in_=ot[:],
                        )

    nc.compile()
    return nc


def get_nc():
    if "nc" not in _CACHE:
        _CACHE["nc"] = build()
    return _CACHE["nc"]


def make_in_maps(x, wq, wk, wv, wo):
    bf = ml_dtypes.bfloat16
    x = np.asarray(x, dtype=np.float32)
    xT = np.ascontiguousarray(x.reshape(TT, D).T.astype(bf))
    woT = np.ascontiguousarray(np.asarray(wo, np.float32).T.astype(bf))
    in_maps = []
    for i in range(NCORES):
        sl = slice(i * OC, (i + 1) * OC)
        in_maps.append({
            "xt": xT,
            "wqt": np.ascontiguousarray(np.asarray(wq, np.float32)[sl, :].T.astype(bf)),
            "wkt": np.ascontiguousarray(np.asarray(wk, np.float32)[sl, :].T.astype(bf)),
            "wvt": np.ascontiguousarray(np.asarray(wv, np.float32)[sl, :].T.astype(bf)),
            "wot": woT,
        })
    return in_maps


def assemble(results):
    return np.concatenate([results[i]["out"] for i in range(NCORES)], axis=0).reshape(B, T, D)


def kernel(x, wq, wk, wv, wo):
    nc = get_nc()
    in_maps = make_in_maps(x, wq, wk, wv, wo)
    res = run_bass_kernel_spmd(nc, in_maps, list(range(NCORES)), trace=False)
    return assemble(res.results)


if __name__ == "__main__":
    rng = np.random.default_rng(0)
    s = 1.0 / math.sqrt(D)
    x = rng.standard_normal((B, T, D), dtype=np.float32)
    wq = (rng.standard_normal((D, D), dtype=np.float32) * s)
    wk = (rng.standard_normal((D, D), dtype=np.float32) * s)
    wv = (rng.standard_normal((D, D), dtype=np.float32) * s)
    wo = (rng.standard_normal((D, D), dtype=np.float32) * s)
    out = kernel(x, wq, wk, wv, wo)
    print("out", out.shape, out.dtype, np.abs(out).mean())
